# revision 1
# baseline (speedup 1.0000x reference)
"""Trainium2 Bass kernel for nn_DilatedConvModel (retrieval_knn).

Model: eeg [B,T,64] -> 1x1 conv (64->8) -> dilated conv stack (8->16->16->16,
dilations 1,3,9, VALID, relu); stimulus [B,S,T,1] -> dilated stack
(1->16->16->16); cosine similarity between all stim/eeg channel pairs over
time; 256->1 linear.  B=64, S=8, T=8192.

Sharding: pure data parallel over B across 8 cores (8 batches per core).

Per-core dataflow (channel-major convs on PE with block-diagonal weights
over the 8 local sequences; bf16 matmuls, fp32 PSUM):
  eeg:  DMA [t,c] tiles -> bf16 -> PE transpose -> conv1x1 -> e1/e2/e3
        (3-tap PSUM accumulation over shifted views, no data copies)
  stim: cast-DMA (f32->bf16) of 3 pre-shifted row blocks -> s1 (shift-stack
        single matmul) -> s2/s3 (3-tap)
  feats: square+sum norms (DVE tensor_tensor_reduce / ACT Square accum),
        normalize in channel-major, zero tails, DMA-xbar transpose to
        [t-part, chunk, ch] layout, dot = PE matmuls contracting t,
        final linear on-device. Output [8,8] fp32 per core.
"""

from contextlib import ExitStack

import numpy as np
import ml_dtypes

import concourse.bass as bass
import concourse.tile as tile
from concourse import mybir
from concourse.bass_utils import run_bass_kernel_spmd
from concourse.vector_clock import ScopedClock

# ---------------------------------------------------------------------------
# Workaround for walrus in this container rejecting >1 sync wait per
# instruction ("Too many sync wait commands"): (a) distribute the
# TileContext tail drain's sem waits across sync-engine nops, (b) post-pass
# that hoists extra waits of any instruction onto standalone EventSemaphore
# instructions inserted just before it on the same engine.
# ---------------------------------------------------------------------------
_MAX_WAITS = 1


def _patched_drain_and_barrier(self, tick_clock, wait_clock):
    nc = self.nc
    probe = nc.sync.nop()
    wait_clock.add_sem_waits(probe.ins,
                             ScopedClock({None: tick_clock.global_clock}))
    si = probe.ins.sync_info
    waits = list(si.on_wait) if si and si.on_wait else []
    if len(waits) > _MAX_WAITS:
        si.on_wait = waits[:_MAX_WAITS]
        rest = waits[_MAX_WAITS:]
        while rest:
            extra = nc.sync.nop()
            extra.ins.sync_info = mybir.SyncInfo(on_wait=rest[:_MAX_WAITS],
                                                 on_update=[])
            rest = rest[_MAX_WAITS:]
    # probe/extra nops precede the drain in SP program order, so the drain
    # only runs after every lane's final tick — no waits needed on it.
    nc.sync.drain()
    nc.all_engine_barrier()
    assert self.sems is not None
    popped = nc._tile_sem_poison_stack.pop()
    assert popped is self._sem_poison
    nc.clear_and_free_semaphores(list(self.sems.allocated().values()))
    nc.all_engine_barrier()


def _split_multi_waits(nc, max_waits=_MAX_WAITS):
    f = nc.m.functions[0]
    ctr = 0
    for bb in f.blocks:
        new_insts = []
        for inst in bb.instructions:
            si = inst.sync_info
            waits = list(si.on_wait) if si and si.on_wait else []
            if len(waits) > max_waits:
                for w in waits[:-max_waits]:
                    ev = mybir.InstEventSemaphore(
                        name=f"waitsplit_{ctr}", opcode="EventSemaphore",
                        engine=inst.engine, ins=[], outs=[],
                        sync_info=mybir.SyncInfo(on_wait=[w], on_update=[]))
                    ctr += 1
                    new_insts.append(ev)
                si.on_wait = waits[-max_waits:]
            new_insts.append(inst)
        try:
            bb.instructions[:] = new_insts
        except TypeError:
            bb.instructions = new_insts


tile.TileContext._drain_and_barrier = _patched_drain_and_barrier

BF16 = mybir.dt.bfloat16
F32 = mybir.dt.float32
AF = mybir.ActivationFunctionType
ALU = mybir.AluOpType

B, S, T, C_EEG = 64, 8, 8192, 64
N_CORES = 8
BPC = B // N_CORES          # 8 sequences per core
CH = 512                    # fp32 PSUM chunk width
L_C1, L_E1, L_E2, L_E3 = 8192, 8190, 8184, 8166
EPS = 1e-8

_NC_CACHE = {}


def _chunks(length):
    out, t0 = [], 0
    while t0 < length:
        w = min(CH, length - t0)
        out.append((t0, w))
        t0 += w
    return out


def _const_shapes():
    d = {
        "Ws1": ((24, 128), BF16),
        "id128": ((128, 128), BF16),
        "id128f": ((128, 128), F32),
        "W2c": ((16, 128), F32),
        "ones16": ((16, 1), F32),
        "ones1x16": ((1, 16), F32),
        "blin": ((1, 1), F32),
    }
    for k in range(3):
        d[f"Wf1_{k}"] = ((128, 32), BF16)
    for l in (2, 3):
        for k in range(3):
            d[f"We{l}_{k}"] = ((128, 128), BF16)
    for l in (2, 3):
        for k in range(3):
            d[f"Ws{l}_{k}"] = ((128, 128), BF16)
    for n in ("bias_e1", "bias_e2", "bias_e3", "bias_s1", "bias_s2",
              "bias_s3"):
        d[n] = ((128, 1), F32)
    return d


def _blob_layout():
    """column layout of consts inside the two blobs (rows, cols, dtype)"""
    bf_items, f32_items = [], []
    for name, (shape, dt) in _const_shapes().items():
        (bf_items if dt == BF16 else f32_items).append((name, shape))
    off_bf, off_f = 0, 0
    lay = {}
    for name, shape in bf_items:
        lay[name] = ("bf", off_bf, shape)
        off_bf += shape[1]
    for name, shape in f32_items:
        lay[name] = ("f32", off_f, shape)
        off_f += shape[1]
    return lay, off_bf, off_f


def _build_body(nc, tc, dram):
    eeg_in, stim_in, out_dram = dram["eeg_in"], dram["stim_in"], dram["out"]
    shapes = _const_shapes()

    with ExitStack() as ctx:
        const_p = ctx.enter_context(tc.tile_pool(name="const", bufs=1))
        persist_p = ctx.enter_context(tc.tile_pool(name="persist", bufs=1))
        early_p = ctx.enter_context(tc.tile_pool(name="early", bufs=1))
        psC_p = ctx.enter_context(tc.tile_pool(name="psC", bufs=3,
                                               space="PSUM"))

        lay, nbf, nf = _blob_layout()
        blob_bf = const_p.tile([128, nbf], BF16, name="blob_bf")
        nc.sync.dma_start(blob_bf[:], dram["blob_bf"][:])
        blob_f32 = const_p.tile([128, nf], F32, name="blob_f32")
        nc.sync.dma_start(blob_f32[:], dram["blob_f32"][:])

        def cload(name):
            which, off, shape = lay[name]
            blob = blob_bf if which == "bf" else blob_f32
            return blob[0:shape[0], off:off + shape[1]]

        Wf1 = [cload(f"Wf1_{k}") for k in range(3)]
        We = {l: [cload(f"We{l}_{k}") for k in range(3)] for l in (2, 3)}
        Ws1 = cload("Ws1")
        Ws = {l: [cload(f"Ws{l}_{k}") for k in range(3)] for l in (2, 3)}
        id128 = cload("id128")
        id128f = cload("id128f")
        bias = {n: cload(n) for n in
                ("bias_e1", "bias_e2", "bias_e3",
                 "bias_s1", "bias_s2", "bias_s3")}
        W2c = cload("W2c")
        ones16 = cload("ones16")
        ones1x16 = cload("ones1x16")
        blin = cload("blin")

        out_sb = const_p.tile([1, BPC * S], F32, name="out_sb")
        inv_nx = const_p.tile([128, 1], F32, name="inv_nx")
        sqscr = const_p.tile([128, T], BF16, name="sqscr")

        xT = persist_p.tile([128, T // 128, 128], BF16, name="xT")
        xf = persist_p.tile([128, T], BF16, name="xf")

        evac_ctr = [0]

        def evac_relu(dst, src, bias_t):
            if evac_ctr[0] % 7 < 4:
                nc.vector.tensor_scalar(dst, src, bias_t[:, 0:1], 0.0,
                                        ALU.add, ALU.max)
            else:
                nc.scalar.activation(dst, src, AF.Relu, bias=bias_t[:, 0:1])
            evac_ctr[0] += 1

        def conv_layer(src_m, rows, dst_m, out_len, dil, Wk, bn):
            # pairs of 512-wide chunks share one [128, 1024] 2-bank psum
            chs = _chunks(out_len)
            for i in range(0, len(chs), 2):
                grp = chs[i:i + 2]
                ps = psC_p.tile([128, 2 * CH], F32, name="psconv",
                                tag="psconv")
                for k in range(3):
                    for j, (t0, w) in enumerate(grp):
                        nc.tensor.matmul(
                            ps[:, j * CH:j * CH + w], Wk[k][:],
                            src_m[0:rows, t0 + k * dil:t0 + k * dil + w],
                            start=(k == 0), stop=(k == 2))
                t0 = grp[0][0]
                wtot = CH + grp[1][1] if len(grp) == 2 else grp[0][1]
                evac_relu(dst_m[:, t0:t0 + wtot], ps[:, :wtot], bias[bn])

        # ---- early: stimulus group 0 s1+s2 (fills PE while eeg DMA runs)
        s1mov = early_p.tile([24, T], BF16, name="s1mov")
        s2in = early_p.tile([128, L_E1], BF16, name="s2in")
        s3in = early_p.tile([128, L_E2], BF16, name="s3in")

        def stim_s1(g):
            for k in range(3):
                nc.gpsimd.dma_start(s1mov[k * 8:(k + 1) * 8, 0:L_E1],
                                    stim_in[g, :, k:k + L_E1])
            chs = _chunks(L_E1)
            for i in range(0, len(chs), 2):
                grp = chs[i:i + 2]
                ps = psC_p.tile([128, 2 * CH], F32, name="psconv",
                                tag="psconv")
                for j, (t0, w) in enumerate(grp):
                    nc.tensor.matmul(ps[:, j * CH:j * CH + w], Ws1[:],
                                     s1mov[0:24, t0:t0 + w])
                t0 = grp[0][0]
                wtot = CH + grp[1][1] if len(grp) == 2 else grp[0][1]
                evac_relu(s2in[:, t0:t0 + wtot], ps[:, :wtot],
                          bias["bias_s1"])

        stim_s1(0)
        conv_layer(s2in, 128, s3in, L_E2, 3, Ws[2], "bias_s2")

        # ================= EEG path =================
        with ExitStack() as ectx:
            eeg_p = ectx.enter_context(tc.tile_pool(name="eegp", bufs=1))
            rot_p = ectx.enter_context(tc.tile_pool(name="eegrot", bufs=5))
            psT_p = ectx.enter_context(tc.tile_pool(name="psT", bufs=2,
                                                    space="PSUM"))

            e2in = eeg_p.tile([128, L_E1], BF16, name="e2in")
            e3in = eeg_p.tile([128, L_E2], BF16, name="e3in")

            TB = 2048
            chs_e1 = _chunks(L_E1)
            for duo in range(2):
                eegT = {}
                for lp in range(2):
                    p = 2 * duo + lp
                    eegT_p = eeg_p.tile([128, T], BF16, name="eegT",
                                        tag=f"eegT_{lp}")
                    eegT[lp] = eegT_p
                    for tb in range(T // TB):
                        ebf = rot_p.tile([128, TB // 128, 2, 64], BF16,
                                         name="ebf")
                        for dlt in range(2):
                            srcd = eeg_in[2 * p + dlt,
                                          tb * TB:(tb + 1) * TB, :]
                            nc.gpsimd.dma_start(
                                ebf[:, :, dlt, :],
                                srcd.rearrange("(th tl) c -> tl th c",
                                               tl=128))
                        for qb in range(TB // (2 * CH)):
                            psT = psT_p.tile([128, 8, 128], BF16,
                                             name="psT")
                            for u in range(8):
                                nc.tensor.matmul(psT[:, u, :],
                                                 ebf[:, qb * 8 + u, :, :],
                                                 id128[:],
                                                 is_transpose=True,
                                                 start=(u == 0),
                                                 stop=(u == 7))
                            t0 = tb * TB + qb * 2 * CH
                            if evac_ctr[0] % 7 < 4:
                                nc.vector.tensor_copy(
                                    eegT_p[:, t0:t0 + 2 * CH], psT[:])
                            else:
                                nc.scalar.copy(
                                    eegT_p[:, t0:t0 + 2 * CH], psT[:])
                            evac_ctr[0] += 1
                # fused conv1x1+e1: 2 pairs on col strips 0/32, own banks
                for i in range(0, len(chs_e1), 2):
                    grp = chs_e1[i:i + 2]
                    t0 = grp[0][0]
                    wtot = CH + grp[1][1] if len(grp) == 2 else grp[0][1]
                    for lp in range(2):
                        ps = psC_p.tile([128, 2 * CH], F32, name="pse1",
                                        tag="psconv")
                        for k in range(3):
                            for j, (tj, w) in enumerate(grp):
                                nc.tensor.matmul(
                                    ps[32 * lp:32 * lp + 32,
                                       j * CH:j * CH + w],
                                    Wf1[k][:],
                                    eegT[lp][:, tj + k:tj + k + w],
                                    start=(k == 0), stop=(k == 2),
                                    tile_position=(0, 32 * lp))
                        r0 = 64 * duo + 32 * lp
                        evac_relu(e2in[r0:r0 + 32, t0:t0 + wtot],
                                  ps[32 * lp:32 * lp + 32, :wtot],
                                  bias["bias_e1"][r0:r0 + 32])

            conv_layer(e2in, 128, e3in, L_E2, 3, We[2], "bias_e2")
            conv_layer(e3in, 128, xf, L_E3, 9, We[3], "bias_e3")

        # ================= stimulus path =================
        with ExitStack() as sctx:
            stim_p = sctx.enter_context(tc.tile_pool(name="stimp", bufs=1))
            stT_p = sctx.enter_context(tc.tile_pool(name="stTp", bufs=2))
            psD_p = sctx.enter_context(tc.tile_pool(name="psD", bufs=1,
                                                    space="PSUM"))
            psF_p = sctx.enter_context(tc.tile_pool(name="psF", bufs=1,
                                                    space="PSUM"))

            pending = []

            def emit_dot(g, stT, invns_row):
                dot_ps = psD_p.tile([16, 128], F32, name="dot_ps")
                NCHK = T // 128
                for c in range(NCHK):
                    nc.tensor.matmul(dot_ps[:],
                                     xT[:, c, g * 16:(g + 1) * 16],
                                     stT[:, c, :], start=(c == 0),
                                     stop=(c == NCHK - 1))
                f1 = const_p.tile([16, 128], F32, name="f1",
                                  tag=f"f1_{g % 2}")
                nc.vector.tensor_mul(f1[:], dot_ps[:], W2c[:])
                psB = psF_p.tile([16, 128], F32, name="psB", tag="psN")
                nc.tensor.matmul(psB[:], ones1x16[:], invns_row[:])
                nc.vector.tensor_mul(f1[:], f1[:], psB[:])
                f3 = const_p.tile([16, S], F32, name="f3", tag=f"f3_{g % 2}")
                nc.vector.tensor_reduce(
                    f3[:], f1.rearrange("p (s i) -> p s i", i=16),
                    mybir.AxisListType.X, ALU.add)
                fin_ps = psF_p.tile([1, S], F32, name="fin_ps", tag="psN")
                nc.tensor.matmul(fin_ps[:], ones16[:], f3[:])
                nc.vector.tensor_scalar_add(
                    out_sb[0:1, g * S:(g + 1) * S], fin_ps[:],
                    blin[0:1, 0:1])

            for g in range(BPC):
                if g > 0:
                    stim_s1(g)
                    conv_layer(s2in, 128, s3in, L_E2, 3, Ws[2], "bias_s2")
                st_cm = stim_p.tile([128, T], BF16, name="st_cm", bufs=2)
                conv_layer(s3in, 128, st_cm, L_E3, 9, Ws[3], "bias_s3")
                if g == 0:
                    # x norms + normalize + zero tail + transpose
                    nx2 = const_p.tile([128, 1], F32, name="nx2")
                    nx4 = const_p.tile([128, 4], F32, name="nx4")
                    qs = (L_E3 + 3) // 4
                    for q in range(4):
                        a, b = q * qs, min((q + 1) * qs, L_E3)
                        nc.scalar.activation(sqscr[:, a:b], xf[:, a:b],
                                             AF.Square,
                                             accum_out=nx4[:, q:q + 1])
                    nc.vector.tensor_reduce(nx2[:], nx4[:],
                                            mybir.AxisListType.X, ALU.add)
                    nc.scalar.sqrt(inv_nx[:], nx2[:])
                    nc.vector.tensor_scalar_max(inv_nx[:], inv_nx[:], EPS)
                    nc.vector.reciprocal(inv_nx[:], inv_nx[:])
                    nc.vector.tensor_scalar_mul(xf[:, :L_E3], xf[:, :L_E3],
                                                inv_nx[:, 0:1])
                    nc.gpsimd.memset(xf[:, L_E3:T], 0.0)
                    for qq in range(4):
                        nc.sync.dma_start_transpose(
                            xT[:, qq * 16:(qq + 1) * 16, :],
                            xf[:, qq * (T // 4):(qq + 1) * (T // 4)])

                ns2 = const_p.tile([128, 1], F32, name="ns2",
                                   tag=f"ns2_{g % 2}")
                ns4 = const_p.tile([128, 4], F32, name="ns4",
                                   tag=f"ns4_{g % 2}")
                qs = (L_E3 + 3) // 4
                for q in range(4):
                    a, b = q * qs, min((q + 1) * qs, L_E3)
                    nc.scalar.activation(sqscr[:, a:b], st_cm[:, a:b],
                                         AF.Square,
                                         accum_out=ns4[:, q:q + 1])
                nc.vector.tensor_reduce(ns2[:], ns4[:],
                                        mybir.AxisListType.X, ALU.add)
                inv_ns = const_p.tile([128, 1], F32, name="inv_ns",
                                      tag=f"invns_{g % 2}")
                nc.scalar.sqrt(inv_ns[:], ns2[:])
                nc.vector.tensor_scalar_max(inv_ns[:], inv_ns[:], EPS)
                nc.vector.reciprocal(inv_ns[:], inv_ns[:])
                psN = psF_p.tile([1, 128], F32, name="psN", tag="psN")
                nc.tensor.matmul(psN[:], inv_ns[:], id128f[:],
                                 is_transpose=True)
                invns_row = const_p.tile([1, 128], F32, name="invns_row",
                                         tag=f"invrow_{g % 2}")
                nc.vector.tensor_copy(invns_row[:], psN[:])
                nc.gpsimd.memset(st_cm[:, L_E3:T], 0.0)

                stT = stT_p.tile([128, T // 128, 128], BF16, name="stT")
                for qq in range(4):
                    nc.sync.dma_start_transpose(
                        stT[:, qq * 16:(qq + 1) * 16, :],
                        st_cm[:, qq * (T // 4):(qq + 1) * (T // 4)])
                pending.append((g, stT, invns_row))
                if len(pending) > 1:
                    emit_dot(*pending.pop(0))
            while pending:
                emit_dot(*pending.pop(0))

        nc.sync.dma_start(out_dram[:], out_sb[:])


def _build(reps=1):
    nc = bass.Bass()
    dram = {
        "eeg_in": nc.dram_tensor("eeg_in", [BPC, T, C_EEG], F32,
                                 kind="ExternalInput"),
        "stim_in": nc.dram_tensor("stim_in", [BPC, S, T], F32,
                                  kind="ExternalInput"),
    }
    lay, nbf, nf = _blob_layout()
    dram["blob_bf"] = nc.dram_tensor("blob_bf", [128, nbf], BF16,
                                     kind="ExternalInput")
    dram["blob_f32"] = nc.dram_tensor("blob_f32", [128, nf], F32,
                                      kind="ExternalInput")
    dram["out"] = nc.dram_tensor("out", [1, BPC * S], F32,
                                 kind="ExternalOutput")

    with tile.TileContext(nc) as tc:
        if reps == 1:
            _build_body(nc, tc, dram)
        else:
            with tc.For_i(0, reps, 1):
                _build_body(nc, tc, dram)
    _split_multi_waits(nc)
    return nc


def _make_consts(inp):
    bf = ml_dtypes.bfloat16
    c = {}
    w_eeg = np.asarray(inp["w_eeg"], np.float32)      # [8, 64, 1]

    def blockdiag(w, n_seq, ci, co):
        out = []
        for k in range(3):
            m = np.zeros((n_seq * ci, n_seq * co), np.float32)
            for s in range(n_seq):
                m[s * ci:(s + 1) * ci, s * co:(s + 1) * co] = w[:, :, k].T
            out.append(m.astype(bf))
        return out

    # fused conv1x1 + e1: Wf[co, c, k] = sum_ci w_e1[co,ci,k] * w_eeg[ci,c]
    w_e1 = np.asarray(inp["w_e1"], np.float32)
    for k in range(3):
        Mk = w_e1[:, :, k] @ w_eeg[:, :, 0]           # [16 co, 64 c]
        m = np.zeros((128, 32), np.float32)
        for s in range(2):
            m[s * 64:(s + 1) * 64, s * 16:(s + 1) * 16] = Mk.T
        c[f"Wf1_{k}"] = m.astype(bf)
    for l, wn in {2: "w_e2", 3: "w_e3"}.items():
        mats = blockdiag(np.asarray(inp[wn], np.float32), 8, 16, 16)
        for k in range(3):
            c[f"We{l}_{k}"] = mats[k]
    for l, wn in {2: "w_s2", 3: "w_s3"}.items():
        mats = blockdiag(np.asarray(inp[wn], np.float32), 8, 16, 16)
        for k in range(3):
            c[f"Ws{l}_{k}"] = mats[k]
    w_s1 = np.asarray(inp["w_s1"], np.float32)        # [16, 1, 3]
    Ws1 = np.zeros((24, 128), np.float32)
    for k in range(3):
        for s in range(8):
            Ws1[k * 8 + s, s * 16:(s + 1) * 16] = w_s1[:, 0, k]
    c["Ws1"] = Ws1.astype(bf)
    c["id128"] = np.eye(128, dtype=np.float32).astype(bf)
    c["id128f"] = np.eye(128, dtype=np.float32)
    b_e1f = (np.asarray(inp["b_e1"], np.float32) +
             sum(w_e1[:, :, k] for k in range(3))
             @ np.asarray(inp["b_eeg"], np.float32))
    c["bias_e1"] = np.tile(b_e1f, 8)[:, None]
    for n, srcn in (("bias_e2", "b_e2"),
                    ("bias_e3", "b_e3"), ("bias_s1", "b_s1"),
                    ("bias_s2", "b_s2"), ("bias_s3", "b_s3")):
        c[n] = np.tile(np.asarray(inp[srcn], np.float32), 8)[:, None]
    w_lin = np.asarray(inp["w_lin"], np.float32).reshape(16, 16)  # [i, j]
    W2c = np.zeros((16, 128), np.float32)
    for s in range(8):
        W2c[:, s * 16:(s + 1) * 16] = w_lin.T                     # [j, i]
    c["W2c"] = W2c
    c["ones16"] = np.ones((16, 1), np.float32)
    c["ones1x16"] = np.ones((1, 16), np.float32)
    c["blin"] = np.asarray(inp["b_lin"], np.float32).reshape(1, 1)
    lay, nbf, nf = _blob_layout()
    bb = np.zeros((128, nbf), ml_dtypes.bfloat16)
    bf32 = np.zeros((128, nf), np.float32)
    for name, (which, off, shape) in lay.items():
        dst = bb if which == "bf" else bf32
        dst[0:shape[0], off:off + shape[1]] = c[name]
    return {"blob_bf": bb, "blob_f32": bf32}


def get_nc(reps=1):
    if reps not in _NC_CACHE:
        _NC_CACHE[reps] = _build(reps)
    return _NC_CACHE[reps]


def run(inputs, reps=1, trace=False):
    nc = get_nc(reps)
    consts = _make_consts(inputs)
    eeg = np.asarray(inputs["eeg"], np.float32)              # [64, 8192, 64]
    stim = np.asarray(inputs["stimulus"], np.float32)[..., 0]  # [64, 8, 8192]
    in_maps = []
    for ci in range(N_CORES):
        m = {"eeg_in": np.ascontiguousarray(eeg[ci * BPC:(ci + 1) * BPC]),
             "stim_in": np.ascontiguousarray(stim[ci * BPC:(ci + 1) * BPC])}
        m.update(consts)
        in_maps.append(m)
    res = run_bass_kernel_spmd(nc, in_maps, list(range(N_CORES)),
                               trace=trace)
    out = np.concatenate(
        [res.results[i]["out"].reshape(BPC, S) for i in range(N_CORES)],
        axis=0)
    return out.astype(np.float32)


def kernel(**inputs):
    return run(inputs, reps=1)

